# revision 28
# baseline (speedup 1.0000x reference)
"""Trainium2 Bass kernel for nn_BaseQuantizer (VQ codebook quantizer).

Data-parallel over batch: 1 batch row (2048 tokens) per NeuronCore, 8 cores.
argmin_n dist(x, cb_n)  ==  argmax_n (x . cb_n - 0.5*|cb_n|^2).

Per core:
  * Codebook resident in SBUF as interleaved bf16 hi/lo pairs [128, 4, N, 2]
    (d = c*128 + p); the -0.5|cb_n|^2 bias is split 3-way into bf16 (h+m+l,
    err ~4e-6) and seeds every PSUM accumulation group via a K=3 ones matmul.
  * Approximate scores: ONE full-rate bf16 pass (x_hi . cb_hi) + exact bias,
    accumulated into 4 PSUM pieces of 2048; DVE Max8/MaxIndex scan each piece
    directly from PSUM (scans interleaved with piece matmuls - trace order
    defines the psum-reuse dependency).
  * Candidates: per-piece top-3 (true argmax is empirically within the global
    top-3 of the bf16 approx, hence within piece top-3); candidate vectors
    gathered from the SBUF codebook via gpsimd ap_gather (16-wrapped index
    list via a small DRAM bounce).
  * Exact rescore: correction matmuls corr = x_hi . c_lo + x_lo . c_hi over
    the tile's 1536 candidate columns; per-token own-candidate corrections
    extracted with a diagonal DRAM-bounce AP; exact12 = approx_top3 + corr;
    winner picked with a masked-min (ties resolve to the lowest code id,
    matching argmin).  Net score precision ~= fp32 (0/16384 index flips).
  * Dequantize: ap_gather of winner hi/lo + reconstruct; commitment/codebook
    SSE accumulated on-device (ACT Square accum); x_st written via PE
    transposes.  3-deep software pipeline: A(t) | corr(t-1) | B(t-2).
Host does only cross-shard reductions: concat, bincount of the device-computed
indices, EMA, loss normalization (per the data-parallel sharding strategy).
"""
import numpy as np

import concourse.bass as bass
import concourse.tile as tile
from concourse import bacc, mybir
from concourse import bass_utils

F32 = mybir.dt.float32
U16 = mybir.dt.uint16
I16 = mybir.dt.int16
U32 = mybir.dt.uint32
U8 = mybir.dt.uint8
BF16 = mybir.dt.bfloat16
AF = mybir.ActivationFunctionType
ALU = mybir.AluOpType

B, L, D, N = 8, 2048, 512, 8192
KC = D // 128            # 4 contraction chunks of 128
NTILES = L // 128        # 16 token tiles per core
ALPHA = 0.95

# argmax pieces: 4 logical pieces x 2048 scores, alternating between two
# 4-bank PSUM regions (psA/psB)
NPIECE = 4
PIECE = N // NPIECE      # 2048


def build_nc():
    nc = bacc.Bacc("TRN2", target_bir_lowering=False, debug=False,
                   enable_asserts=False, num_devices=8)

    # ---------------- DRAM ----------------
    x_d = nc.dram_tensor("x", [L, D], F32, kind="ExternalInput").ap()
    cb_d = nc.dram_tensor("cb", [D, N], F32, kind="ExternalInput").ap()
    nbias_d = nc.dram_tensor("nbias", [N], F32, kind="ExternalInput").ap()  # -0.5*|cb_n|^2
    nbh_d = nc.dram_tensor("nbh_scratch", [3, N], BF16, kind="Internal").ap()
    cand_d = nc.dram_tensor("cand_scratch", [NTILES, 128 * 12], U16, kind="Internal").ap()
    corr_d = nc.dram_tensor("corr_scratch", [NTILES, 128 * 128 * 12], F32, kind="Internal").ap()

    xst_d = nc.dram_tensor("xst", [L, D], F32, kind="ExternalOutput").ap()
    idx_d = nc.dram_tensor("idx", [L], U16, kind="ExternalOutput").ap()
    sse_d = nc.dram_tensor("sse", [1, 1], F32, kind="ExternalOutput").ap()

    # ---------------- SBUF (persistent) ----------------
    cb_hl = nc.alloc_sbuf_tensor("cb_hl", [128, KC, N, 2], BF16).ap()  # d = c*128+p; [...,0]=hi [...,1]=lo
    ident = nc.alloc_sbuf_tensor("ident", [128, 128], F32).ap()
    iota_r = nc.alloc_sbuf_tensor("iota_r", [128, 128], F32).ap()
    iota_c = nc.alloc_sbuf_tensor("iota_c", [128, 1], F32).ap()
    ones_col = nc.alloc_sbuf_tensor("ones_col", [128, 1], F32).ap()
    ones_k3 = nc.alloc_sbuf_tensor("ones_k3", [3, 128], BF16).ap()
    bias_hml = nc.alloc_sbuf_tensor("bias_hml", [3, N], BF16).ap()
    nb32 = nc.alloc_sbuf_tensor("nb32", [128, N // 128], F32).ap()
    nbtmp = nc.alloc_sbuf_tensor("nbtmp", [128, N // 128, 3], BF16).ap()
    nbr = nc.alloc_sbuf_tensor("nbr", [128, N // 128], F32).ap()
    ssev = nc.alloc_sbuf_tensor("ssev", [128, NTILES], F32).ap()
    sse_sb = nc.alloc_sbuf_tensor("sse_sb", [1, 1], F32).ap()

    # ---------------- PSUM ----------------
    # two physical regions of 4 banks; the 4 logical score pieces alternate
    # between them; transposes + sse reduce time-share the same regions
    # (scheduler serializes via WAR/WAW deps).
    psA = nc.alloc_psum_tensor("psA", [128, PIECE], F32).ap()
    psB = nc.alloc_psum_tensor("psB", [128, PIECE], F32).ap()
    ps = [psA, psB, psA, psB]

    with tile.TileContext(nc) as tc:
        # ---- constants ----
        nc.gpsimd.iota(iota_r[:], pattern=[[1, 128]], base=0, channel_multiplier=0,
                       allow_small_or_imprecise_dtypes=True)
        nc.gpsimd.iota(iota_c[:], pattern=[[0, 1]], base=0, channel_multiplier=1,
                       allow_small_or_imprecise_dtypes=True)
        nc.vector.tensor_scalar(ident[:], iota_r[:], iota_c[:], None, ALU.is_equal)
        nc.vector.memset(ones_col[:], 1.0)
        nc.gpsimd.memset(ones_k3[:], 1.0)

        # ---- 3-way bf16 split of nbias (wide layout, then bounce to [3, N]) ----
        nc.sync.dma_start(nb32[:], nbias_d.rearrange("(p j) -> p j", p=128))
        nc.scalar.copy(nbtmp[:, :, 0], nb32[:])                      # hi
        nc.vector.tensor_sub(nbr[:], nb32[:], nbtmp[:, :, 0])        # r1 = b - hi (f32)
        nc.scalar.copy(nbtmp[:, :, 1], nbr[:])                       # mid
        nc.vector.tensor_sub(nbtmp[:, :, 2], nbr[:], nbtmp[:, :, 1])  # lo (bf16)
        with nc.allow_non_contiguous_dma(reason="small bias bounce"):
            for r in range(3):
                nc.sync.dma_start(nbh_d[r].rearrange("(p j) -> p j", p=128), nbtmp[:, :, r])
        nc.sync.dma_start(bias_hml[:], nbh_d[:])

        # ---- load + split codebook into bf16 hi/lo ----
        cb_r = cb_d.rearrange("(c p) n -> p c n", c=KC, p=128)
        with tc.tile_pool(name="cbstage", bufs=2) as cbst_pool:
            for h in range(4):
                for c in range(KC):
                    sl = slice(h * 2048, (h + 1) * 2048)
                    cbst = cbst_pool.tile([128, 2048], F32, tag="cbst")
                    nc.sync.dma_start(cbst[:], cb_r[:, c, sl])
                    nc.scalar.copy(cb_hl[:, c, sl, 0], cbst[:])                      # f32 -> bf16 (hi)
                    nc.vector.tensor_sub(cb_hl[:, c, sl, 1], cbst[:], cb_hl[:, c, sl, 0])  # f32 - bf16 -> bf16 (lo)

        # ---- tile pools for the main loop ----
        with (
            tc.tile_pool(name="xrow", bufs=2) as xrow_pool,
            tc.tile_pool(name="xt", bufs=3) as xt_pool,
            tc.tile_pool(name="small", bufs=4) as small_pool,
            tc.tile_pool(name="gath", bufs=2) as gath_pool,
            tc.tile_pool(name="cand", bufs=2) as cand_pool,
            tc.tile_pool(name="outp", bufs=2) as out_pool,
            tc.tile_pool(name="corrp", bufs=1) as corr_pool,
            tc.tile_pool(name="stg", bufs=3) as stg_pool,
        ):
            NC12 = 128 * 12
            state = {}

            def stage_a(t):
                tok = slice(t * 128, (t + 1) * 128)
                # ---- load x tile and transpose to [d, tok] ----
                xr = xrow_pool.tile([128, D], F32, tag="xr")
                nc.sync.dma_start(xr[:], x_d[tok, :])
                xt32 = xt_pool.tile([128, KC, 128], F32, tag="xt32")
                for c in range(KC):
                    pst = psA[:, c * 128:(c + 1) * 128]
                    nc.tensor.transpose(pst, xr[:, c * 128:(c + 1) * 128], ident[:])
                    nc.scalar.copy(xt32[:, c, :], pst)
                xhi = xt_pool.tile([128, KC, 128], BF16, tag="xhi")
                xlo = xt_pool.tile([128, KC, 128], BF16, tag="xlo")
                nc.scalar.copy(xhi[:], xt32[:])
                nc.vector.tensor_sub(xlo[:], xt32[:], xhi[:])

                # ---- approx scores (1 bf16 pass + exact bias seed) + piece scans ----
                mx8 = small_pool.tile([128, NPIECE, 8], F32, tag="mx8")
                ix8 = small_pool.tile([128, NPIECE, 8], U32, tag="ix8")
                for pc in range(NPIECE):
                    for nci in range(PIECE // 512):
                        n0 = pc * PIECE + nci * 512
                        outap = ps[pc][:, nci * 512:(nci + 1) * 512]
                        stg = stg_pool.tile([128, KC, 512], BF16, tag="stg")
                        nc.scalar.copy(stg[:], cb_hl[:, :, n0:n0 + 512, 0])
                        nc.tensor.matmul(outap, ones_k3[:], bias_hml[:, n0:n0 + 512],
                                         start=True, stop=False)
                        for kc in range(KC):
                            nc.tensor.matmul(
                                outap, xhi[:, kc, :], stg[:, kc, :],
                                start=False, stop=(kc == KC - 1),
                            )
                    nc.vector.max(mx8[:, pc, :], ps[pc][:])
                    nc.vector.max_index(ix8[:, pc, :], mx8[:, pc, :], ps[pc][:])
                for pc in range(1, NPIECE):
                    nc.vector.tensor_scalar(ix8[:, pc, :], ix8[:, pc, :], float(pc * PIECE), None, ALU.add)

                # ---- candidates: per-piece top-3, bounce + wrapped + hi/lo gathers ----
                cand12 = small_pool.tile([128, 12], U16, tag="cand12")
                nc.vector.tensor_copy(cand12[:].rearrange("p (a s) -> p a s", s=3), ix8[:, :, 0:3])
                nc.sync.dma_start(cand_d[t].rearrange("(p s) -> p s", p=128), cand12[:])
                candw = small_pool.tile([128, 96], I16, tag="candw")
                with nc.allow_non_contiguous_dma(reason="small wrapped cand load"):
                    nc.sync.dma_start(
                        candw[:].rearrange("(g k) m -> g k m", g=8),
                        bass.AP(tensor=cand_d.tensor, offset=t * 1536,
                                ap=[[0, 8], [1, 16], [16, 96]]).bitcast(I16))
                state[t] = (xt32, xhi, xlo, mx8, cand12, candw)

            def stage_corr(t):
                # corr = xhi . c_lo + xlo . c_hi  (2 bf16 passes, no bias) -> psA
                xt32, xhi, xlo, mx8, cand12, candw = state[t]
                for c in range(KC):
                    chl = cand_pool.tile([128, NC12, 2], BF16, tag="chl")
                    nc.gpsimd.ap_gather(
                        chl[:], cb_hl[:, c, :, :], candw[:],
                        channels=128, num_elems=N, d=2, num_idxs=NC12)
                    for ch in range(3):
                        outap = psA[:, ch * 512:(ch + 1) * 512]
                        sl = slice(ch * 512, (ch + 1) * 512)
                        nc.tensor.matmul(outap, xhi[:, c, :], chl[:, sl, 1],
                                         start=(c == 0), stop=False)
                        nc.tensor.matmul(outap, xlo[:, c, :], chl[:, sl, 0],
                                         start=False, stop=(c == KC - 1))
                corr_sb = corr_pool.tile([128, NC12], F32, tag="corr_sb")
                for half in range(2):
                    nc.scalar.copy(corr_sb[:, half * 768:(half + 1) * 768],
                                   psA[:, half * 768:(half + 1) * 768])
                nc.sync.dma_start(corr_d[t], corr_sb[:])

            def stage_b(t):
                tok = slice(t * 128, (t + 1) * 128)
                xt32, xhi, xlo, mx8, cand12, candw = state.pop(t)
                # ---- own-diagonal + resolve ----
                own12 = small_pool.tile([128, 12], F32, tag="own12")
                nc.sync.dma_start(
                    own12[:],
                    bass.AP(tensor=corr_d.tensor, offset=t * (128 * NC12 // 128) * 128,
                            ap=[[12 * 129, 128], [1, 12]]))
                exact12 = small_pool.tile([128, 12], F32, tag="exact12")
                nc.vector.tensor_add(exact12[:].rearrange("p (a s) -> p a s", s=3),
                                     mx8[:, :, 0:3], own12[:].rearrange("p (a s) -> p a s", s=3))
                candf = small_pool.tile([128, 12], F32, tag="candf")
                nc.vector.tensor_copy(candf[:], cand12[:])
                em = small_pool.tile([128, 1], F32, tag="em")
                nc.vector.tensor_reduce(em[:], exact12[:], axis=mybir.AxisListType.X, op=ALU.max)
                emask = small_pool.tile([128, 12], F32, tag="emask")
                nc.vector.tensor_single_scalar(emask[:], exact12[:], em[:], ALU.is_ge)
                nc.vector.tensor_scalar(emask[:], emask[:], -1.0, 1.0, ALU.add, ALU.mult)
                nc.vector.tensor_scalar(emask[:], emask[:], -65535.0, None, ALU.mult)
                nc.vector.tensor_tensor(emask[:], candf[:], emask[:], ALU.add)
                idx16 = small_pool.tile([128, 1], U16, tag="idx16")
                em2 = small_pool.tile([128, 1], F32, tag="em2")
                nc.vector.tensor_reduce(em2[:], emask[:], axis=mybir.AxisListType.X, op=ALU.min)
                nc.vector.tensor_copy(idx16[:], em2[:])

                # ---- idx bounce + final gathers ----
                nc.sync.dma_start(idx_d.rearrange("(tt p) -> p tt", p=128)[:, t:t + 1], idx16[:])
                idxw = small_pool.tile([128, 8], I16, tag="idxw")
                with nc.allow_non_contiguous_dma(reason="256B wrapped idx load"):
                    nc.sync.dma_start(
                        idxw[:].rearrange("(g k) m -> g k m", g=8),
                        bass.AP(tensor=idx_d.tensor, offset=t * 128,
                                ap=[[0, 8], [1, 16], [16, 8]]).bitcast(I16))
                ghl = gath_pool.tile([128, KC, 128, 2], BF16, tag="ghl")
                for c in range(KC):
                    nc.gpsimd.ap_gather(
                        ghl[:, c, :, :], cb_hl[:, c, :, :],
                        idxw[:], channels=128, num_elems=N, d=2, num_idxs=128)
                xq32 = gath_pool.tile([128, KC, 128], F32, tag="xq32")
                nc.gpsimd.tensor_add(xq32[:], ghl[:, :, :, 0], ghl[:, :, :, 1])

                # ---- transpose xq -> [tok, d] (psA tail region), write x_st ----
                xst = out_pool.tile([128, D], F32, tag="xst")
                for c in range(KC):
                    pst = psA[:, 1536:1664]
                    nc.tensor.transpose(pst, xq32[:, c, :], ident[:])
                    nc.scalar.copy(xst[:, c * 128:(c + 1) * 128], pst)
                nc.sync.dma_start(xst_d[tok, :], xst[:])

                # ---- SSE: in-place diff + square-accumulate ----
                nc.gpsimd.tensor_sub(xq32[:], xq32[:], xt32[:])
                nc.scalar.activation(xq32[:], xq32[:], AF.Square, accum_out=ssev[:, t:t + 1])

            # ---- 3-deep software pipeline ----
            for t in range(NTILES):
                stage_a(t)
                if t >= 1:
                    stage_corr(t - 1)
                if t >= 2:
                    stage_b(t - 2)
            stage_corr(NTILES - 1)
            stage_b(NTILES - 2)
            stage_b(NTILES - 1)

            # ---- final SSE reduction: sum ssev over tiles then partitions ----
            ssetot = small_pool.tile([128, 1], F32, tag="ssetot")
            nc.vector.tensor_reduce(ssetot[:], ssev[:], axis=mybir.AxisListType.X, op=ALU.add)
            nc.tensor.matmul(psA[0:1, 0:1], ssetot[:], ones_col[:], start=True, stop=True)
            nc.scalar.copy(sse_sb[:], psA[0:1, 0:1])
            nc.sync.dma_start(sse_d[:], sse_sb[:])

    nc.compile()
    return nc


_NC_CACHE = None


def _get_nc():
    global _NC_CACHE
    if _NC_CACHE is None:
        _NC_CACHE = build_nc()
    return _NC_CACHE


def kernel(x_in: np.ndarray, codebook: np.ndarray, cluster_frequency: np.ndarray):
    assert x_in.shape == (B, L, D) and codebook.shape == (D, N)
    nc = _get_nc()
    x_in = np.ascontiguousarray(x_in, dtype=np.float32)
    codebook = np.ascontiguousarray(codebook, dtype=np.float32)

    nbias = (-0.5 * (codebook.astype(np.float64) ** 2).sum(0)).astype(np.float32)
    in_maps = [{"x": x_in[b], "cb": codebook, "nbias": nbias} for b in range(B)]
    try:
        res = bass_utils.run_bass_kernel_spmd(nc, in_maps, core_ids=list(range(B)))
    except Exception:
        # transient NRT device hiccups have been observed; retry once
        import time as _time
        _time.sleep(10)
        res = bass_utils.run_bass_kernel_spmd(nc, in_maps, core_ids=list(range(B)))

    xst = np.stack([res.results[b]["xst"] for b in range(B)])           # [B, L, D]
    idx = np.stack([res.results[b]["idx"].astype(np.int32) for b in range(B)])  # [B, L]
    sse = np.array([res.results[b]["sse"].ravel()[0] for b in range(B)])

    # host-side cross-shard reductions (per sharding strategy)
    inner_loss = np.float32(2.0 * (np.float64(sse.sum()) / (B * L * D)))
    counts = np.bincount(idx.reshape(-1), minlength=N).astype(np.float32)
    new_cf = (np.float32(ALPHA) * cluster_frequency.astype(np.float32)
              + np.float32(1.0 - ALPHA) * counts)
    return xst, idx, inner_loss, new_cf


# revision 29
# speedup vs baseline: 1.1109x; 1.1109x over previous
"""Trainium2 Bass kernel for nn_BaseQuantizer (VQ codebook quantizer).

Data-parallel over batch: 1 batch row (2048 tokens) per NeuronCore, 8 cores.
argmin_n dist(x, cb_n)  ==  argmax_n (x . cb_n - 0.5*|cb_n|^2).

Per core:
  * Codebook resident in SBUF as interleaved bf16 hi/lo pairs [128, 4, N, 2]
    (d = c*128 + p); the -0.5|cb_n|^2 bias is split 3-way into bf16 (h+m+l,
    err ~4e-6) and seeds every PSUM accumulation group via a K=3 ones matmul.
  * Approximate scores: ONE full-rate bf16 pass (x_hi . cb_hi) + exact bias,
    accumulated into 4 PSUM pieces of 2048; DVE Max8/MaxIndex scan each piece
    directly from PSUM (scans interleaved with piece matmuls - trace order
    defines the psum-reuse dependency).
  * Candidates: per-piece top-3 (true argmax is empirically within the global
    top-3 of the bf16 approx, hence within piece top-3); candidate vectors
    gathered from the SBUF codebook via gpsimd ap_gather (16-wrapped index
    list via a small DRAM bounce).
  * Exact rescore: correction matmuls corr = x_hi . c_lo + x_lo . c_hi over
    the tile's 1536 candidate columns; per-token own-candidate corrections
    extracted with a diagonal DRAM-bounce AP; exact12 = approx_top3 + corr;
    winner picked with a masked-min (ties resolve to the lowest code id,
    matching argmin).  Net score precision ~= fp32 (0/16384 index flips).
  * Dequantize: ap_gather of winner hi/lo + reconstruct; commitment/codebook
    SSE accumulated on-device (ACT Square accum); x_st written via PE
    transposes.  3-deep software pipeline: A(t) | corr(t-1) | B(t-2).
Host does only cross-shard reductions: concat, bincount of the device-computed
indices, EMA, loss normalization (per the data-parallel sharding strategy).
"""
import numpy as np

import concourse.bass as bass
import concourse.tile as tile
from concourse import bacc, mybir
from concourse import bass_utils

F32 = mybir.dt.float32
U16 = mybir.dt.uint16
I16 = mybir.dt.int16
U32 = mybir.dt.uint32
U8 = mybir.dt.uint8
BF16 = mybir.dt.bfloat16
AF = mybir.ActivationFunctionType
ALU = mybir.AluOpType

B, L, D, N = 8, 2048, 512, 8192
KC = D // 128            # 4 contraction chunks of 128
NTILES = L // 128        # 16 token tiles per core
ALPHA = 0.95

# argmax pieces: 4 logical pieces x 2048 scores, alternating between two
# 4-bank PSUM regions (psA/psB)
NPIECE = 4
PIECE = N // NPIECE      # 2048


def build_nc():
    nc = bacc.Bacc("TRN2", target_bir_lowering=False, debug=False,
                   enable_asserts=False, num_devices=8)

    # ---------------- DRAM ----------------
    x_d = nc.dram_tensor("x", [L, D], F32, kind="ExternalInput").ap()
    cb_d = nc.dram_tensor("cb", [D, N], F32, kind="ExternalInput").ap()
    nbias_d = nc.dram_tensor("nbias", [N], F32, kind="ExternalInput").ap()  # -0.5*|cb_n|^2
    nbh_d = nc.dram_tensor("nbh_scratch", [3, N], BF16, kind="Internal").ap()
    cand_d = nc.dram_tensor("cand_scratch", [NTILES, 128 * 12], U16, kind="Internal").ap()
    corr_d = nc.dram_tensor("corr_scratch", [NTILES, 128 * 128 * 12], F32, kind="Internal").ap()

    xst_d = nc.dram_tensor("xst", [L, D], F32, kind="ExternalOutput").ap()
    idx_d = nc.dram_tensor("idx", [L], U16, kind="ExternalOutput").ap()
    sse_d = nc.dram_tensor("sse", [1, 1], F32, kind="ExternalOutput").ap()

    # ---------------- SBUF (persistent) ----------------
    cb_hl = nc.alloc_sbuf_tensor("cb_hl", [128, KC, N, 2], BF16).ap()  # d = c*128+p; [...,0]=hi [...,1]=lo
    ident = nc.alloc_sbuf_tensor("ident", [128, 128], F32).ap()
    iota_r = nc.alloc_sbuf_tensor("iota_r", [128, 128], F32).ap()
    iota_c = nc.alloc_sbuf_tensor("iota_c", [128, 1], F32).ap()
    ones_col = nc.alloc_sbuf_tensor("ones_col", [128, 1], F32).ap()
    ones_k3 = nc.alloc_sbuf_tensor("ones_k3", [3, 128], BF16).ap()
    bias_hml = nc.alloc_sbuf_tensor("bias_hml", [3, N], BF16).ap()
    nb32 = nc.alloc_sbuf_tensor("nb32", [128, N // 128], F32).ap()
    nbtmp = nc.alloc_sbuf_tensor("nbtmp", [128, N // 128, 3], BF16).ap()
    nbr = nc.alloc_sbuf_tensor("nbr", [128, N // 128], F32).ap()
    ssev = nc.alloc_sbuf_tensor("ssev", [128, NTILES], F32).ap()
    sse_sb = nc.alloc_sbuf_tensor("sse_sb", [1, 1], F32).ap()

    # ---------------- PSUM ----------------
    # two physical regions of 4 banks; the 4 logical score pieces alternate
    # between them; transposes + sse reduce time-share the same regions
    # (scheduler serializes via WAR/WAW deps).
    psA = nc.alloc_psum_tensor("psA", [128, PIECE], F32).ap()
    psB = nc.alloc_psum_tensor("psB", [128, PIECE], F32).ap()
    ps = [psA, psB, psA, psB]

    with tile.TileContext(nc) as tc:
        # ---- constants ----
        nc.gpsimd.iota(iota_r[:], pattern=[[1, 128]], base=0, channel_multiplier=0,
                       allow_small_or_imprecise_dtypes=True)
        nc.gpsimd.iota(iota_c[:], pattern=[[0, 1]], base=0, channel_multiplier=1,
                       allow_small_or_imprecise_dtypes=True)
        nc.vector.tensor_scalar(ident[:], iota_r[:], iota_c[:], None, ALU.is_equal)
        nc.vector.memset(ones_col[:], 1.0)
        nc.gpsimd.memset(ones_k3[:], 1.0)

        # ---- 3-way bf16 split of nbias (wide layout, then bounce to [3, N]) ----
        nc.sync.dma_start(nb32[:], nbias_d.rearrange("(p j) -> p j", p=128))
        nc.scalar.copy(nbtmp[:, :, 0], nb32[:])                      # hi
        nc.vector.tensor_sub(nbr[:], nb32[:], nbtmp[:, :, 0])        # r1 = b - hi (f32)
        nc.scalar.copy(nbtmp[:, :, 1], nbr[:])                       # mid
        nc.vector.tensor_sub(nbtmp[:, :, 2], nbr[:], nbtmp[:, :, 1])  # lo (bf16)
        with nc.allow_non_contiguous_dma(reason="small bias bounce"):
            for r in range(3):
                nc.sync.dma_start(nbh_d[r].rearrange("(p j) -> p j", p=128), nbtmp[:, :, r])
        nc.sync.dma_start(bias_hml[:], nbh_d[:])

        # ---- load + split codebook into bf16 hi/lo ----
        cb_r = cb_d.rearrange("(c p) n -> p c n", c=KC, p=128)
        with tc.tile_pool(name="cbstage", bufs=2) as cbst_pool:
            for h in range(4):
                for c in range(KC):
                    sl = slice(h * 2048, (h + 1) * 2048)
                    cbst = cbst_pool.tile([128, 2048], F32, tag="cbst")
                    nc.sync.dma_start(cbst[:], cb_r[:, c, sl])
                    nc.scalar.copy(cb_hl[:, c, sl, 0], cbst[:])                      # f32 -> bf16 (hi)
                    nc.vector.tensor_sub(cb_hl[:, c, sl, 1], cbst[:], cb_hl[:, c, sl, 0])  # f32 - bf16 -> bf16 (lo)

        # ---- tile pools for the main loop ----
        with (
            tc.tile_pool(name="xrow", bufs=2) as xrow_pool,
            tc.tile_pool(name="xt", bufs=3) as xt_pool,
            tc.tile_pool(name="small", bufs=4) as small_pool,
            tc.tile_pool(name="gath", bufs=1) as gath_pool,
            tc.tile_pool(name="cand", bufs=2) as cand_pool,
            tc.tile_pool(name="chlp", bufs=1) as chl_pool,
            tc.tile_pool(name="outp", bufs=2) as out_pool,
            tc.tile_pool(name="corrp", bufs=1) as corr_pool,
            tc.tile_pool(name="stg", bufs=2) as stg_pool,
        ):
            NC12 = 128 * 12
            state = {}

            def stage_a(t):
                tok = slice(t * 128, (t + 1) * 128)
                # ---- load x tile and transpose to [d, tok] ----
                xr = xrow_pool.tile([128, D], F32, tag="xr")
                nc.sync.dma_start(xr[:], x_d[tok, :])
                xt32 = xt_pool.tile([128, KC, 128], F32, tag="xt32")
                for c in range(KC):
                    pst = psA[:, c * 128:(c + 1) * 128]
                    nc.tensor.transpose(pst, xr[:, c * 128:(c + 1) * 128], ident[:])
                    nc.scalar.copy(xt32[:, c, :], pst)
                xhi = xt_pool.tile([128, KC, 128], BF16, tag="xhi")
                xlo = xt_pool.tile([128, KC, 128], BF16, tag="xlo")
                nc.scalar.copy(xhi[:], xt32[:])
                nc.vector.tensor_sub(xlo[:], xt32[:], xhi[:])

                # ---- approx scores (1 bf16 pass + exact bias seed) + piece scans ----
                mx8 = small_pool.tile([128, NPIECE, 8], F32, tag="mx8")
                ix8 = small_pool.tile([128, NPIECE, 8], U32, tag="ix8")
                for pc in range(NPIECE):
                    for nci in range(PIECE // 512):
                        n0 = pc * PIECE + nci * 512
                        outap = ps[pc][:, nci * 512:(nci + 1) * 512]
                        stg = stg_pool.tile([128, KC, 512], BF16, tag="stg")
                        nc.scalar.copy(stg[:], cb_hl[:, :, n0:n0 + 512, 0])
                        nc.tensor.matmul(outap, ones_k3[:], bias_hml[:, n0:n0 + 512],
                                         start=True, stop=False)
                        for kc in range(KC):
                            nc.tensor.matmul(
                                outap, xhi[:, kc, :], stg[:, kc, :],
                                start=False, stop=(kc == KC - 1),
                            )
                    nc.vector.max(mx8[:, pc, :], ps[pc][:])
                    nc.vector.max_index(ix8[:, pc, :], mx8[:, pc, :], ps[pc][:])
                for pc in range(1, NPIECE):
                    nc.vector.tensor_scalar(ix8[:, pc, :], ix8[:, pc, :], float(pc * PIECE), None, ALU.add)

                # ---- candidates: per-piece top-3, bounce + wrapped + hi/lo gathers ----
                cand12 = small_pool.tile([128, 12], U16, tag="cand12")
                nc.vector.tensor_copy(cand12[:].rearrange("p (a s) -> p a s", s=3), ix8[:, :, 0:3])
                nc.sync.dma_start(cand_d[t].rearrange("(p s) -> p s", p=128), cand12[:])
                candw = small_pool.tile([128, 96], I16, tag="candw")
                with nc.allow_non_contiguous_dma(reason="small wrapped cand load"):
                    nc.sync.dma_start(
                        candw[:].rearrange("(g k) m -> g k m", g=8),
                        bass.AP(tensor=cand_d.tensor, offset=t * 1536,
                                ap=[[0, 8], [1, 16], [16, 96]]).bitcast(I16))
                state[t] = (xt32, xhi, xlo, mx8, cand12, candw)

            def stage_corr(t):
                # corr = xhi . c_lo + xlo . c_hi  (2 bf16 passes, no bias) -> psA
                xt32, xhi, xlo, mx8, cand12, candw = state[t]
                for c in range(KC):
                    chl = chl_pool.tile([128, NC12, 2], BF16, tag="chl")
                    nc.gpsimd.ap_gather(
                        chl[:], cb_hl[:, c, :, :], candw[:],
                        channels=128, num_elems=N, d=2, num_idxs=NC12)
                    # de-interleave to contiguous planes (strided rhs costs 2x)
                    cpl = cand_pool.tile([128, 2, NC12], BF16, tag="cpl")
                    nc.scalar.copy(cpl[:, 0, :], chl[:, :, 0])
                    nc.scalar.copy(cpl[:, 1, :], chl[:, :, 1])
                    for ch in range(3):
                        outap = psA[:, ch * 512:(ch + 1) * 512]
                        sl = slice(ch * 512, (ch + 1) * 512)
                        nc.tensor.matmul(outap, xhi[:, c, :], cpl[:, 1, sl],
                                         start=(c == 0), stop=False)
                        nc.tensor.matmul(outap, xlo[:, c, :], cpl[:, 0, sl],
                                         start=False, stop=(c == KC - 1))
                corr_sb = corr_pool.tile([128, NC12], F32, tag="corr_sb")
                for half in range(2):
                    nc.scalar.copy(corr_sb[:, half * 768:(half + 1) * 768],
                                   psA[:, half * 768:(half + 1) * 768])
                nc.sync.dma_start(corr_d[t], corr_sb[:])

            def stage_b(t):
                tok = slice(t * 128, (t + 1) * 128)
                xt32, xhi, xlo, mx8, cand12, candw = state.pop(t)
                # ---- own-diagonal + resolve ----
                own12 = small_pool.tile([128, 12], F32, tag="own12")
                nc.sync.dma_start(
                    own12[:],
                    bass.AP(tensor=corr_d.tensor, offset=t * (128 * NC12 // 128) * 128,
                            ap=[[12 * 129, 128], [1, 12]]))
                exact12 = small_pool.tile([128, 12], F32, tag="exact12")
                nc.vector.tensor_add(exact12[:].rearrange("p (a s) -> p a s", s=3),
                                     mx8[:, :, 0:3], own12[:].rearrange("p (a s) -> p a s", s=3))
                candf = small_pool.tile([128, 12], F32, tag="candf")
                nc.vector.tensor_copy(candf[:], cand12[:])
                em = small_pool.tile([128, 1], F32, tag="em")
                nc.vector.tensor_reduce(em[:], exact12[:], axis=mybir.AxisListType.X, op=ALU.max)
                emask = small_pool.tile([128, 12], F32, tag="emask")
                nc.vector.tensor_single_scalar(emask[:], exact12[:], em[:], ALU.is_ge)
                nc.vector.tensor_scalar(emask[:], emask[:], -1.0, 1.0, ALU.add, ALU.mult)
                nc.vector.tensor_scalar(emask[:], emask[:], -65535.0, None, ALU.mult)
                nc.vector.tensor_tensor(emask[:], candf[:], emask[:], ALU.add)
                idx16 = small_pool.tile([128, 1], U16, tag="idx16")
                em2 = small_pool.tile([128, 1], F32, tag="em2")
                nc.vector.tensor_reduce(em2[:], emask[:], axis=mybir.AxisListType.X, op=ALU.min)
                nc.vector.tensor_copy(idx16[:], em2[:])

                # ---- idx bounce + final gathers ----
                nc.sync.dma_start(idx_d.rearrange("(tt p) -> p tt", p=128)[:, t:t + 1], idx16[:])
                idxw = small_pool.tile([128, 8], I16, tag="idxw")
                with nc.allow_non_contiguous_dma(reason="256B wrapped idx load"):
                    nc.sync.dma_start(
                        idxw[:].rearrange("(g k) m -> g k m", g=8),
                        bass.AP(tensor=idx_d.tensor, offset=t * 128,
                                ap=[[0, 8], [1, 16], [16, 8]]).bitcast(I16))
                ghl = gath_pool.tile([128, KC, 128, 2], BF16, tag="ghl")
                for c in range(KC):
                    nc.gpsimd.ap_gather(
                        ghl[:, c, :, :], cb_hl[:, c, :, :],
                        idxw[:], channels=128, num_elems=N, d=2, num_idxs=128)
                xq32 = gath_pool.tile([128, KC, 128], F32, tag="xq32")
                nc.gpsimd.tensor_add(xq32[:], ghl[:, :, :, 0], ghl[:, :, :, 1])

                # ---- transpose xq -> [tok, d] (psA tail region), write x_st ----
                xst = out_pool.tile([128, D], F32, tag="xst")
                for c in range(KC):
                    pst = psA[:, 1536:1664]
                    nc.tensor.transpose(pst, xq32[:, c, :], ident[:])
                    nc.scalar.copy(xst[:, c * 128:(c + 1) * 128], pst)
                nc.sync.dma_start(xst_d[tok, :], xst[:])

                # ---- SSE: in-place diff + square-accumulate ----
                nc.gpsimd.tensor_sub(xq32[:], xq32[:], xt32[:])
                nc.scalar.activation(xq32[:], xq32[:], AF.Square, accum_out=ssev[:, t:t + 1])

            # ---- 3-deep software pipeline ----
            for t in range(NTILES):
                stage_a(t)
                if t >= 1:
                    stage_corr(t - 1)
                if t >= 2:
                    stage_b(t - 2)
            stage_corr(NTILES - 1)
            stage_b(NTILES - 2)
            stage_b(NTILES - 1)

            # ---- final SSE reduction: sum ssev over tiles then partitions ----
            ssetot = small_pool.tile([128, 1], F32, tag="ssetot")
            nc.vector.tensor_reduce(ssetot[:], ssev[:], axis=mybir.AxisListType.X, op=ALU.add)
            nc.tensor.matmul(psA[0:1, 0:1], ssetot[:], ones_col[:], start=True, stop=True)
            nc.scalar.copy(sse_sb[:], psA[0:1, 0:1])
            nc.sync.dma_start(sse_d[:], sse_sb[:])

    nc.compile()
    return nc


_NC_CACHE = None


def _get_nc():
    global _NC_CACHE
    if _NC_CACHE is None:
        _NC_CACHE = build_nc()
    return _NC_CACHE


def kernel(x_in: np.ndarray, codebook: np.ndarray, cluster_frequency: np.ndarray):
    assert x_in.shape == (B, L, D) and codebook.shape == (D, N)
    nc = _get_nc()
    x_in = np.ascontiguousarray(x_in, dtype=np.float32)
    codebook = np.ascontiguousarray(codebook, dtype=np.float32)

    nbias = (-0.5 * (codebook.astype(np.float64) ** 2).sum(0)).astype(np.float32)
    in_maps = [{"x": x_in[b], "cb": codebook, "nbias": nbias} for b in range(B)]
    try:
        res = bass_utils.run_bass_kernel_spmd(nc, in_maps, core_ids=list(range(B)))
    except Exception:
        # transient NRT device hiccups have been observed; retry once
        import time as _time
        _time.sleep(10)
        res = bass_utils.run_bass_kernel_spmd(nc, in_maps, core_ids=list(range(B)))

    xst = np.stack([res.results[b]["xst"] for b in range(B)])           # [B, L, D]
    idx = np.stack([res.results[b]["idx"].astype(np.int32) for b in range(B)])  # [B, L]
    sse = np.array([res.results[b]["sse"].ravel()[0] for b in range(B)])

    # host-side cross-shard reductions (per sharding strategy)
    inner_loss = np.float32(2.0 * (np.float64(sse.sum()) / (B * L * D)))
    counts = np.bincount(idx.reshape(-1), minlength=N).astype(np.float32)
    new_cf = (np.float32(ALPHA) * cluster_frequency.astype(np.float32)
              + np.float32(1.0 - ALPHA) * counts)
    return xst, idx, inner_loss, new_cf


# revision 30
# speedup vs baseline: 1.1739x; 1.0567x over previous
"""Trainium2 Bass kernel for nn_BaseQuantizer (VQ codebook quantizer).

Data-parallel over batch: 1 batch row (2048 tokens) per NeuronCore, 8 cores.
argmin_n dist(x, cb_n)  ==  argmax_n (x . cb_n - 0.5*|cb_n|^2).

Per core:
  * Codebook resident in SBUF as interleaved bf16 hi/lo pairs [128, 4, N, 2]
    (d = c*128 + p); the -0.5|cb_n|^2 bias is split 3-way into bf16 (h+m+l,
    err ~4e-6) and seeds every PSUM accumulation group via a K=3 ones matmul.
  * Approximate scores: ONE full-rate bf16 pass (x_hi . cb_hi) + exact bias,
    accumulated into 4 PSUM pieces of 2048; DVE Max8/MaxIndex scan each piece
    directly from PSUM (scans interleaved with piece matmuls - trace order
    defines the psum-reuse dependency).
  * Candidates: per-piece top-3 (true argmax is empirically within the global
    top-3 of the bf16 approx, hence within piece top-3); candidate vectors
    gathered from the SBUF codebook via gpsimd ap_gather (16-wrapped index
    list via a small DRAM bounce).
  * Exact rescore: correction matmuls corr = x_hi . c_lo + x_lo . c_hi over
    the tile's 1536 candidate columns; per-token own-candidate corrections
    extracted with a diagonal DRAM-bounce AP; exact12 = approx_top3 + corr;
    winner picked with a masked-min (ties resolve to the lowest code id,
    matching argmin).  Net score precision ~= fp32 (0/16384 index flips).
  * Dequantize: ap_gather of winner hi/lo + reconstruct; commitment/codebook
    SSE accumulated on-device (ACT Square accum); x_st written via PE
    transposes.  3-deep software pipeline: A(t) | corr(t-1) | B(t-2).
Host does only cross-shard reductions: concat, bincount of the device-computed
indices, EMA, loss normalization (per the data-parallel sharding strategy).
"""
import numpy as np

import concourse.bass as bass
import concourse.tile as tile
from concourse import bacc, mybir
from concourse import bass_utils

F32 = mybir.dt.float32
U16 = mybir.dt.uint16
I16 = mybir.dt.int16
U32 = mybir.dt.uint32
U8 = mybir.dt.uint8
BF16 = mybir.dt.bfloat16
AF = mybir.ActivationFunctionType
ALU = mybir.AluOpType

B, L, D, N = 8, 2048, 512, 8192
KC = D // 128            # 4 contraction chunks of 128
NTILES = L // 128        # 16 token tiles per core
ALPHA = 0.95

# argmax pieces: 4 logical pieces x 2048 scores, alternating between two
# 4-bank PSUM regions (psA/psB)
NPIECE = 4
PIECE = N // NPIECE      # 2048


def build_nc():
    nc = bacc.Bacc("TRN2", target_bir_lowering=False, debug=False,
                   enable_asserts=False, num_devices=8)

    # ---------------- DRAM ----------------
    x_d = nc.dram_tensor("x", [L, D], F32, kind="ExternalInput").ap()
    cb_d = nc.dram_tensor("cb", [D, N], F32, kind="ExternalInput").ap()
    nbias_d = nc.dram_tensor("nbias", [N], F32, kind="ExternalInput").ap()  # -0.5*|cb_n|^2
    nbh_d = nc.dram_tensor("nbh_scratch", [3, N], BF16, kind="Internal").ap()
    cand_d = nc.dram_tensor("cand_scratch", [NTILES, 128 * 4], U16, kind="Internal").ap()
    corr_d = nc.dram_tensor("corr_scratch", [NTILES, 128 * 128 * 4], F32, kind="Internal").ap()

    xst_d = nc.dram_tensor("xst", [L, D], F32, kind="ExternalOutput").ap()
    idx_d = nc.dram_tensor("idx", [L], U16, kind="ExternalOutput").ap()
    sse_d = nc.dram_tensor("sse", [1, 1], F32, kind="ExternalOutput").ap()

    # ---------------- SBUF (persistent) ----------------
    cb_hl = nc.alloc_sbuf_tensor("cb_hl", [128, KC, N, 2], BF16).ap()  # d = c*128+p; [...,0]=hi [...,1]=lo
    ident = nc.alloc_sbuf_tensor("ident", [128, 128], F32).ap()
    iota_r = nc.alloc_sbuf_tensor("iota_r", [128, 128], F32).ap()
    iota_c = nc.alloc_sbuf_tensor("iota_c", [128, 1], F32).ap()
    iota32 = nc.alloc_sbuf_tensor("iota32", [128, 32], F32).ap()
    ones_col = nc.alloc_sbuf_tensor("ones_col", [128, 1], F32).ap()
    ones_k3 = nc.alloc_sbuf_tensor("ones_k3", [3, 128], BF16).ap()
    bias_hml = nc.alloc_sbuf_tensor("bias_hml", [3, N], BF16).ap()
    nb32 = nc.alloc_sbuf_tensor("nb32", [128, N // 128], F32).ap()
    nbtmp = nc.alloc_sbuf_tensor("nbtmp", [128, N // 128, 3], BF16).ap()
    nbr = nc.alloc_sbuf_tensor("nbr", [128, N // 128], F32).ap()
    ssev = nc.alloc_sbuf_tensor("ssev", [128, NTILES], F32).ap()
    sse_sb = nc.alloc_sbuf_tensor("sse_sb", [1, 1], F32).ap()

    # ---------------- PSUM ----------------
    # two physical regions of 4 banks; the 4 logical score pieces alternate
    # between them; transposes + sse reduce time-share the same regions
    # (scheduler serializes via WAR/WAW deps).
    psA = nc.alloc_psum_tensor("psA", [128, PIECE], F32).ap()
    psB = nc.alloc_psum_tensor("psB", [128, PIECE], F32).ap()
    ps = [psA, psB, psA, psB]

    with tile.TileContext(nc) as tc:
        # ---- constants ----
        nc.gpsimd.iota(iota_r[:], pattern=[[1, 128]], base=0, channel_multiplier=0,
                       allow_small_or_imprecise_dtypes=True)
        nc.gpsimd.iota(iota_c[:], pattern=[[0, 1]], base=0, channel_multiplier=1,
                       allow_small_or_imprecise_dtypes=True)
        nc.vector.tensor_scalar(ident[:], iota_r[:], iota_c[:], None, ALU.is_equal)
        nc.gpsimd.iota(iota32[:], pattern=[[1, 32]], base=0, channel_multiplier=0,
                       allow_small_or_imprecise_dtypes=True)
        nc.vector.memset(ones_col[:], 1.0)
        nc.gpsimd.memset(ones_k3[:], 1.0)

        # ---- 3-way bf16 split of nbias (wide layout, then bounce to [3, N]) ----
        nc.sync.dma_start(nb32[:], nbias_d.rearrange("(p j) -> p j", p=128))
        nc.scalar.copy(nbtmp[:, :, 0], nb32[:])                      # hi
        nc.vector.tensor_sub(nbr[:], nb32[:], nbtmp[:, :, 0])        # r1 = b - hi (f32)
        nc.scalar.copy(nbtmp[:, :, 1], nbr[:])                       # mid
        nc.vector.tensor_sub(nbtmp[:, :, 2], nbr[:], nbtmp[:, :, 1])  # lo (bf16)
        with nc.allow_non_contiguous_dma(reason="small bias bounce"):
            for r in range(3):
                nc.sync.dma_start(nbh_d[r].rearrange("(p j) -> p j", p=128), nbtmp[:, :, r])
        nc.sync.dma_start(bias_hml[:], nbh_d[:])

        # ---- load + split codebook into bf16 hi/lo ----
        cb_r = cb_d.rearrange("(c p) n -> p c n", c=KC, p=128)
        with tc.tile_pool(name="cbstage", bufs=2) as cbst_pool:
            for h in range(4):
                for c in range(KC):
                    sl = slice(h * 2048, (h + 1) * 2048)
                    cbst = cbst_pool.tile([128, 2048], F32, tag="cbst")
                    nc.sync.dma_start(cbst[:], cb_r[:, c, sl])
                    nc.scalar.copy(cb_hl[:, c, sl, 0], cbst[:])                      # f32 -> bf16 (hi)
                    nc.vector.tensor_sub(cb_hl[:, c, sl, 1], cbst[:], cb_hl[:, c, sl, 0])  # f32 - bf16 -> bf16 (lo)

        # ---- tile pools for the main loop ----
        with (
            tc.tile_pool(name="xrow", bufs=2) as xrow_pool,
            tc.tile_pool(name="xt", bufs=3) as xt_pool,
            tc.tile_pool(name="small", bufs=4) as small_pool,
            tc.tile_pool(name="gath", bufs=1) as gath_pool,
            tc.tile_pool(name="cand", bufs=2) as cand_pool,
            tc.tile_pool(name="chlp", bufs=1) as chl_pool,
            tc.tile_pool(name="outp", bufs=2) as out_pool,
            tc.tile_pool(name="corrp", bufs=1) as corr_pool,
            tc.tile_pool(name="stg", bufs=2) as stg_pool,
        ):
            NCAND = 128 * 4
            state = {}

            def stage_a(t):
                tok = slice(t * 128, (t + 1) * 128)
                # ---- load x tile and transpose to [d, tok] ----
                xr = xrow_pool.tile([128, D], F32, tag="xr")
                nc.sync.dma_start(xr[:], x_d[tok, :])
                xt32 = xt_pool.tile([128, KC, 128], F32, tag="xt32")
                for c in range(KC):
                    pst = psA[:, c * 128:(c + 1) * 128]
                    nc.tensor.transpose(pst, xr[:, c * 128:(c + 1) * 128], ident[:])
                    nc.scalar.copy(xt32[:, c, :], pst)
                xhi = xt_pool.tile([128, KC, 128], BF16, tag="xhi")
                xlo = xt_pool.tile([128, KC, 128], BF16, tag="xlo")
                nc.scalar.copy(xhi[:], xt32[:])
                nc.vector.tensor_sub(xlo[:], xt32[:], xhi[:])

                # ---- approx scores (1 bf16 pass + exact bias seed) + piece scans ----
                mx8 = small_pool.tile([128, NPIECE, 8], F32, tag="mx8")
                ix8 = small_pool.tile([128, NPIECE, 8], U32, tag="ix8")
                for pc in range(NPIECE):
                    for nci in range(PIECE // 512):
                        n0 = pc * PIECE + nci * 512
                        outap = ps[pc][:, nci * 512:(nci + 1) * 512]
                        stg = stg_pool.tile([128, KC, 512], BF16, tag="stg")
                        nc.scalar.copy(stg[:], cb_hl[:, :, n0:n0 + 512, 0])
                        nc.tensor.matmul(outap, ones_k3[:], bias_hml[:, n0:n0 + 512],
                                         start=True, stop=False)
                        for kc in range(KC):
                            nc.tensor.matmul(
                                outap, xhi[:, kc, :], stg[:, kc, :],
                                start=False, stop=(kc == KC - 1),
                            )
                    nc.vector.max(mx8[:, pc, :], ps[pc][:])
                    nc.vector.max_index(ix8[:, pc, :], mx8[:, pc, :], ps[pc][:])
                for pc in range(1, NPIECE):
                    nc.vector.tensor_scalar(ix8[:, pc, :], ix8[:, pc, :], float(pc * PIECE), None, ALU.add)

                # ---- candidates: per-piece top-3, bounce + wrapped + hi/lo gathers ----
                cand12 = small_pool.tile([128, 12], U16, tag="cand12")
                nc.vector.tensor_copy(cand12[:].rearrange("p (a s) -> p a s", s=3), ix8[:, :, 0:3])
                nc.sync.dma_start(cand_d[t].rearrange("(p s) -> p s", p=128), cand12[:])
                candw = small_pool.tile([128, 96], I16, tag="candw")
                with nc.allow_non_contiguous_dma(reason="small wrapped cand load"):
                    nc.sync.dma_start(
                        candw[:].rearrange("(g k) m -> g k m", g=8),
                        bass.AP(tensor=cand_d.tensor, offset=t * 1536,
                                ap=[[0, 8], [1, 16], [16, 96]]).bitcast(I16))
                state[t] = (xt32, xhi, xlo, mx8, cand12, candw)

            def stage_corr(t):
                # corr = xhi . c_lo + xlo . c_hi  (2 bf16 passes, no bias) -> psA
                xt32, xhi, xlo, gm8, cand4, candw = state[t]
                outap = psA[:, 0:NCAND]
                for c in range(KC):
                    chl = chl_pool.tile([128, NCAND, 2], BF16, tag="chl")
                    nc.gpsimd.ap_gather(
                        chl[:], cb_hl[:, c, :, :], candw[:],
                        channels=128, num_elems=N, d=2, num_idxs=NCAND)
                    # de-interleave to contiguous planes (strided rhs costs 2x)
                    cpl = cand_pool.tile([128, 2, NCAND], BF16, tag="cpl")
                    nc.scalar.copy(cpl[:, 0, :], chl[:, :, 0])
                    nc.scalar.copy(cpl[:, 1, :], chl[:, :, 1])
                    nc.tensor.matmul(outap, xhi[:, c, :], cpl[:, 1, :],
                                     start=(c == 0), stop=False)
                    nc.tensor.matmul(outap, xlo[:, c, :], cpl[:, 0, :],
                                     start=False, stop=(c == KC - 1))
                corr_sb = corr_pool.tile([128, NCAND], F32, tag="corr_sb")
                nc.scalar.copy(corr_sb[:], outap)
                nc.sync.dma_start(corr_d[t], corr_sb[:])

            def stage_b(t):
                tok = slice(t * 128, (t + 1) * 128)
                xt32, xhi, xlo, gm8, cand4, candw = state.pop(t)
                # ---- own-diagonal + resolve ----
                own4 = small_pool.tile([128, 4], F32, tag="own4")
                nc.sync.dma_start(
                    own4[:],
                    bass.AP(tensor=corr_d.tensor, offset=t * 128 * NCAND,
                            ap=[[4 * 129, 128], [1, 4]]))
                exact4 = small_pool.tile([128, 4], F32, tag="exact4")
                nc.vector.tensor_add(exact4[:], gm8[:, 0:4], own4[:])
                candf = small_pool.tile([128, 4], F32, tag="candf")
                nc.vector.tensor_copy(candf[:], cand4[:])
                em = small_pool.tile([128, 1], F32, tag="em")
                nc.vector.tensor_reduce(em[:], exact4[:], axis=mybir.AxisListType.X, op=ALU.max)
                emask = small_pool.tile([128, 4], F32, tag="emask")
                nc.vector.tensor_single_scalar(emask[:], exact4[:], em[:], ALU.is_ge)
                nc.vector.tensor_scalar(emask[:], emask[:], -1.0, 1.0, ALU.add, ALU.mult)
                nc.vector.tensor_scalar(emask[:], emask[:], -65535.0, None, ALU.mult)
                nc.vector.tensor_tensor(emask[:], candf[:], emask[:], ALU.add)
                idx16 = small_pool.tile([128, 1], U16, tag="idx16")
                em2 = small_pool.tile([128, 1], F32, tag="em2")
                nc.vector.tensor_reduce(em2[:], emask[:], axis=mybir.AxisListType.X, op=ALU.min)
                nc.vector.tensor_copy(idx16[:], em2[:])

                # ---- idx bounce + final gathers ----
                nc.sync.dma_start(idx_d.rearrange("(tt p) -> p tt", p=128)[:, t:t + 1], idx16[:])
                idxw = small_pool.tile([128, 8], I16, tag="idxw")
                with nc.allow_non_contiguous_dma(reason="256B wrapped idx load"):
                    nc.sync.dma_start(
                        idxw[:].rearrange("(g k) m -> g k m", g=8),
                        bass.AP(tensor=idx_d.tensor, offset=t * 128,
                                ap=[[0, 8], [1, 16], [16, 8]]).bitcast(I16))
                ghl = gath_pool.tile([128, KC, 128, 2], BF16, tag="ghl")
                for c in range(KC):
                    nc.gpsimd.ap_gather(
                        ghl[:, c, :, :], cb_hl[:, c, :, :],
                        idxw[:], channels=128, num_elems=N, d=2, num_idxs=128)
                xq32 = gath_pool.tile([128, KC, 128], F32, tag="xq32")
                nc.gpsimd.tensor_add(xq32[:], ghl[:, :, :, 0], ghl[:, :, :, 1])

                # ---- transpose xq -> [tok, d] (psA tail region), write x_st ----
                xst = out_pool.tile([128, D], F32, tag="xst")
                for c in range(KC):
                    pst = psA[:, 1536:1664]
                    nc.tensor.transpose(pst, xq32[:, c, :], ident[:])
                    nc.scalar.copy(xst[:, c * 128:(c + 1) * 128], pst)
                nc.sync.dma_start(xst_d[tok, :], xst[:])

                # ---- SSE: in-place diff + square-accumulate ----
                nc.gpsimd.tensor_sub(xq32[:], xq32[:], xt32[:])
                nc.scalar.activation(xq32[:], xq32[:], AF.Square, accum_out=ssev[:, t:t + 1])

            # ---- 3-deep software pipeline ----
            for t in range(NTILES):
                stage_a(t)
                if t >= 1:
                    stage_corr(t - 1)
                if t >= 2:
                    stage_b(t - 2)
            stage_corr(NTILES - 1)
            stage_b(NTILES - 2)
            stage_b(NTILES - 1)

            # ---- final SSE reduction: sum ssev over tiles then partitions ----
            ssetot = small_pool.tile([128, 1], F32, tag="ssetot")
            nc.vector.tensor_reduce(ssetot[:], ssev[:], axis=mybir.AxisListType.X, op=ALU.add)
            nc.tensor.matmul(psA[0:1, 0:1], ssetot[:], ones_col[:], start=True, stop=True)
            nc.scalar.copy(sse_sb[:], psA[0:1, 0:1])
            nc.sync.dma_start(sse_d[:], sse_sb[:])

    nc.compile()
    return nc


_NC_CACHE = None


def _get_nc():
    global _NC_CACHE
    if _NC_CACHE is None:
        _NC_CACHE = build_nc()
    return _NC_CACHE


def kernel(x_in: np.ndarray, codebook: np.ndarray, cluster_frequency: np.ndarray):
    assert x_in.shape == (B, L, D) and codebook.shape == (D, N)
    nc = _get_nc()
    x_in = np.ascontiguousarray(x_in, dtype=np.float32)
    codebook = np.ascontiguousarray(codebook, dtype=np.float32)

    nbias = (-0.5 * (codebook.astype(np.float64) ** 2).sum(0)).astype(np.float32)
    in_maps = [{"x": x_in[b], "cb": codebook, "nbias": nbias} for b in range(B)]
    try:
        res = bass_utils.run_bass_kernel_spmd(nc, in_maps, core_ids=list(range(B)))
    except Exception:
        # transient NRT device hiccups have been observed; retry once
        import time as _time
        _time.sleep(10)
        res = bass_utils.run_bass_kernel_spmd(nc, in_maps, core_ids=list(range(B)))

    xst = np.stack([res.results[b]["xst"] for b in range(B)])           # [B, L, D]
    idx = np.stack([res.results[b]["idx"].astype(np.int32) for b in range(B)])  # [B, L]
    sse = np.array([res.results[b]["sse"].ravel()[0] for b in range(B)])

    # host-side cross-shard reductions (per sharding strategy)
    inner_loss = np.float32(2.0 * (np.float64(sse.sum()) / (B * L * D)))
    counts = np.bincount(idx.reshape(-1), minlength=N).astype(np.float32)
    new_cf = (np.float32(ALPHA) * cluster_frequency.astype(np.float32)
              + np.float32(1.0 - ALPHA) * counts)
    return xst, idx, inner_loss, new_cf
